# revision 17
# baseline (speedup 1.0000x reference)
"""Trainium2 kernel for the MeshVerticalLayer problem.

Math: out = (cc_mul(o, diag) + cc_mul(o, off_diag)[..., pp])[..., rp]
with o = x[..., lp], x: [2, B, N] f32 (real/imag stacked on axis 0).

Every output column j depends on exactly two input columns through fixed
complex coefficients, so the whole op is a (very sparse) linear map along
N that is identical for every batch row b.  Strategy:

- Host: transpose x to row-major-[2N, B] layout, group the N output
  columns into T tiles of <=64 columns whose input dependencies close
  under <=64 input columns (works for any permutation `pp`; for the
  pairwise-swap / identity cases this gives exactly T = N/64 = 16 tiles).
  Pre-gather the input rows into tile order, and build per-tile 128x128
  coefficient matrices W (both complex components and both dependency
  columns folded in).
- Device (8 cores, batch-parallel over B): pure streaming
  load -> TensorE matmul(W_t) -> PSUM->SBUF copy -> store.  This is
  memory-bound: ~32MB in + ~32MB out per core.
- Host: inverse-gather rows (folds the right permutation) and transpose
  back to [2, B, N].
"""

import os
import sys

import numpy as np

if "/opt/trn_rl_repo" not in sys.path and os.path.isdir("/opt/trn_rl_repo"):
    sys.path.insert(0, "/opt/trn_rl_repo")

NCORES = 8
FTILE = 2048  # free-dim (batch) chunk per DMA tile
MMF = 512  # matmul moving-dim max / one PSUM bank of fp32

_prog_cache: dict = {}
LAST_RESULTS = None  # BassKernelResults of the most recent device run


def _group_columns(pp: np.ndarray, n: int):
    """Partition output columns [0, n) into blocks of <=64 columns such
    that |block ∪ pp[block]| <= 64.  Walk permutation cycles of pp so the
    union grows by <=1 per added column."""
    visited = np.zeros(n, dtype=bool)
    seq = []
    for s in range(n):
        k = s
        while not visited[k]:
            visited[k] = True
            seq.append(k)
            k = int(pp[k])
    blocks = []
    i = 0
    while i < len(seq):
        block = []
        union = set()
        while i < len(seq) and len(block) < 64:
            k = seq[i]
            new_union = union | {k, int(pp[k])}
            if len(new_union) > 64:
                break
            union = new_union
            block.append(k)
            i += 1
        assert block, "single column exceeded union budget (impossible)"
        blocks.append((block, sorted(union)))
    return blocks


def _build_plan(diag, off_diag, pp, lp, rp, n):
    """Returns (T, W [128, T*128] f32 in lhsT layout, g_in [T*128] row-gather
    indices into the [2N, B] transposed input, g_fin [2N] row-gather indices
    into the device output)."""
    blocks = _group_columns(pp, n)
    T = len(blocks)
    W = np.zeros((128, T * 128), dtype=np.float32)
    g_in = np.zeros(T * 128, dtype=np.int64)
    pos_of_k = np.zeros(n, dtype=np.int64)
    for tt, (outc, inc) in enumerate(blocks):
        idx = {c: i for i, c in enumerate(inc)}
        for r, col in enumerate(inc):
            g_in[tt * 128 + r] = lp[col]  # component 0 rows
            g_in[tt * 128 + 64 + r] = n + lp[col]  # component 1 rows
        for u, k in enumerate(outc):
            p = int(pp[k])
            ik, ip = idx[k], idx[p]
            po0 = tt * 128 + u
            po1 = tt * 128 + 64 + u
            d0, d1 = float(diag[0, k]), float(diag[1, k])
            f0, f1 = float(off_diag[0, p]), float(off_diag[1, p])
            W[ik, po0] += d0
            W[64 + ik, po0] += -d1
            W[ip, po0] += f0
            W[64 + ip, po0] += -f1
            W[ik, po1] += d1
            W[64 + ik, po1] += d0
            W[ip, po1] += f1
            W[64 + ip, po1] += f0
            pos_of_k[k] = tt * 128 + u
    g_fin = np.empty(2 * n, dtype=np.int64)
    g_fin[:n] = pos_of_k[rp]
    g_fin[n:] = pos_of_k[rp] + 64
    return T, W, g_in, g_fin


def _apply_plan_numpy(W, g_in, g_fin, xt, n):
    """Reference emulation of the device program (for plan validation)."""
    T = W.shape[1] // 128
    dev_in = xt[g_in]
    dev_out = np.empty_like(dev_in)
    for tt in range(T):
        wt = W[:, tt * 128 : (tt + 1) * 128]
        dev_out[tt * 128 : (tt + 1) * 128] = wt.T @ dev_in[tt * 128 : (tt + 1) * 128]
    return dev_out[g_fin]


def _build_program(
    T,
    bc,
    ftile=None,
    bufs=4,
    wsplit=False,
    dma_mode="mixed",
    wengine="scalar",
    in_bufs=None,
    out_bufs=None,
    ramp=False,
    copy_eng="both",
):
    import concourse.bacc as bacc
    import concourse.bass as bass
    import concourse.mybir as mybir
    import concourse.tile as tile

    ftile = ftile or FTILE
    while bc % ftile:
        ftile //= 2
    assert ftile % MMF == 0 and bc % ftile == 0, (bc, ftile)
    in_bufs = in_bufs or bufs
    out_bufs = out_bufs or bufs

    def widths_for(tt):
        # Uniform ftile-wide positions, except optionally ramped tile widths
        # at the very start (compute/stores begin sooner -> shorter pipeline
        # fill) and the very end (faster drain of the final stores).
        ws = [ftile] * (bc // ftile)
        if ramp and ftile == 2048 and bc >= 2 * ftile:
            if tt == 0:
                ws = [512, 512, 1024] + [ftile] * ((bc - 2048) // ftile)
            elif tt == T - 1 and ramp is True:  # ramp="up" skips the tail ramp
                ws = [ftile] * ((bc - 2048) // ftile) + [1024, 512, 512]
        return ws

    nc = bacc.Bacc("TRN2", target_bir_lowering=False, debug=False)
    R = T * 128
    a = nc.dram_tensor("a", [R, bc], mybir.dt.float32, kind="ExternalInput")
    w = nc.dram_tensor("w", [128, R], mybir.dt.float32, kind="ExternalInput")
    o = nc.dram_tensor("o", [R, bc], mybir.dt.float32, kind="ExternalOutput")

    with tile.TileContext(nc) as tc:
        with (
            tc.tile_pool(name="wpool", bufs=1) as wpool,
            tc.tile_pool(name="inp", bufs=in_bufs) as inp,
            tc.tile_pool(name="outp", bufs=out_bufs) as outp,
            tc.tile_pool(name="ps", bufs=8, space=bass.MemorySpace.PSUM) as ps,
        ):
            w_s = wpool.tile([128, R], mybir.dt.float32)
            # issue the coefficient load on the store ring (idle at startup)
            # so it doesn't delay the first input-tile load on the sync ring
            w_eng = nc.scalar if wengine == "scalar" else nc.sync
            if wsplit:
                # one DMA per W block so the first matmul only waits on
                # its own 64KB block, not the full 1MB coefficient load
                for tt in range(T):
                    w_eng.dma_start(
                        w_s[:, tt * 128 : (tt + 1) * 128],
                        w[:, tt * 128 : (tt + 1) * 128],
                    )
            else:
                w_eng.dma_start(w_s[:], w[:])
            pos = 0
            for tt in range(T):
                wt = w_s[:, tt * 128 : (tt + 1) * 128]
                c0 = 0
                for width in widths_for(tt):
                    # which HWDGE ring (sync vs scalar engine) issues each DMA
                    if dma_mode == "spread":
                        ld_eng = nc.sync if pos % 2 == 0 else nc.scalar
                        st_eng = nc.scalar if pos % 2 == 0 else nc.sync
                    elif dma_mode == "sync":
                        ld_eng = st_eng = nc.sync
                    else:  # "mixed": loads on sync, stores on scalar
                        ld_eng, st_eng = nc.sync, nc.scalar
                    pos += 1
                    tin = inp.tile([128, width], mybir.dt.float32)
                    ld_eng.dma_start(
                        tin[:],
                        a[tt * 128 : (tt + 1) * 128, c0 : c0 + width],
                    )
                    tout = outp.tile([128, width], mybir.dt.float32)
                    for q in range(width // MMF):
                        pt = ps.tile([128, MMF], mybir.dt.float32)
                        nc.tensor.matmul(
                            pt[:],
                            wt,
                            tin[:, q * MMF : (q + 1) * MMF],
                            start=True,
                            stop=True,
                        )
                        if copy_eng == "dve" or (copy_eng == "both" and q % 2 == 0):
                            nc.vector.tensor_copy(tout[:, q * MMF : (q + 1) * MMF], pt[:])
                        else:
                            nc.scalar.copy(tout[:, q * MMF : (q + 1) * MMF], pt[:])
                    st_eng.dma_start(
                        o[tt * 128 : (tt + 1) * 128, c0 : c0 + width],
                        tout[:],
                    )
                    c0 += width
    nc.compile()
    return nc


# Overridable program-variant knobs (used by experiment sweeps).
PROG_KWARGS: dict = {}


def _get_program(T, bc):
    key = (T, bc, tuple(sorted(PROG_KWARGS.items())))
    if key not in _prog_cache:
        _prog_cache[key] = _build_program(T, bc, **PROG_KWARGS)
    return _prog_cache[key]


def kernel(x, diag, off_diag, pairwise_perm_idx, left_perm_idx, right_perm_idx):
    global LAST_RESULTS
    from concourse.bass_utils import run_bass_kernel_spmd

    x = np.asarray(x)
    in_dtype = x.dtype
    diag = np.asarray(diag, dtype=np.float32)
    off_diag = np.asarray(off_diag, dtype=np.float32)
    pp = np.asarray(pairwise_perm_idx, dtype=np.int64)
    lp = np.asarray(left_perm_idx, dtype=np.int64)
    rp = np.asarray(right_perm_idx, dtype=np.int64)
    _, B, n = x.shape
    bc = B // NCORES

    T, W, g_in, g_fin = _build_plan(diag, off_diag, pp, lp, rp, n)

    # Host-side: transpose to [2N, B] and pre-gather rows into tile order.
    xt = np.ascontiguousarray(x.astype(np.float32, copy=False).transpose(0, 2, 1)).reshape(
        2 * n, B
    )
    dev_in = xt[g_in]  # [T*128, B]

    nc = _get_program(T, bc)
    in_maps = [
        {"a": np.ascontiguousarray(dev_in[:, c * bc : (c + 1) * bc]), "w": W}
        for c in range(NCORES)
    ]
    LAST_RESULTS = run_bass_kernel_spmd(nc, in_maps, list(range(NCORES)))
    dev_out = np.concatenate([LAST_RESULTS.results[c]["o"] for c in range(NCORES)], axis=1)

    out = dev_out[g_fin].reshape(2, n, B).transpose(0, 2, 1)
    return np.ascontiguousarray(out).astype(in_dtype, copy=False)


# revision 21
# speedup vs baseline: 1.0414x; 1.0414x over previous
"""Trainium2 kernel for the MeshVerticalLayer problem.

Math: out = (cc_mul(o, diag) + cc_mul(o, off_diag)[..., pp])[..., rp]
with o = x[..., lp], x: [2, B, N] f32 (real/imag stacked on axis 0).

Every output column j depends on exactly two input columns through fixed
complex coefficients, so the whole op is a (very sparse) linear map along
N that is identical for every batch row b.  Strategy:

- Host: transpose x to row-major-[2N, B] layout, group the N output
  columns into T tiles of <=64 columns whose input dependencies close
  under <=64 input columns (works for any permutation `pp`; for the
  pairwise-swap / identity cases this gives exactly T = N/64 = 16 tiles).
  Pre-gather the input rows into tile order, and build per-tile 128x128
  coefficient matrices W (both complex components and both dependency
  columns folded in).
- Device (8 cores, batch-parallel over B): pure streaming
  load -> TensorE matmul(W_t) -> PSUM->SBUF copy -> store.  This is
  memory-bound: ~32MB in + ~32MB out per core.
- Host: inverse-gather rows (folds the right permutation) and transpose
  back to [2, B, N].
"""

import os
import sys

import numpy as np

if "/opt/trn_rl_repo" not in sys.path and os.path.isdir("/opt/trn_rl_repo"):
    sys.path.insert(0, "/opt/trn_rl_repo")

NCORES = 8
FTILE = 2048  # free-dim (batch) chunk per DMA tile
MMF = 512  # matmul moving-dim max / one PSUM bank of fp32

_prog_cache: dict = {}
LAST_RESULTS = None  # BassKernelResults of the most recent device run


def _group_columns(pp: np.ndarray, n: int):
    """Partition output columns [0, n) into blocks of <=64 columns such
    that |block ∪ pp[block]| <= 64.  Walk permutation cycles of pp so the
    union grows by <=1 per added column."""
    visited = np.zeros(n, dtype=bool)
    seq = []
    for s in range(n):
        k = s
        while not visited[k]:
            visited[k] = True
            seq.append(k)
            k = int(pp[k])
    blocks = []
    i = 0
    while i < len(seq):
        block = []
        union = set()
        while i < len(seq) and len(block) < 64:
            k = seq[i]
            new_union = union | {k, int(pp[k])}
            if len(new_union) > 64:
                break
            union = new_union
            block.append(k)
            i += 1
        assert block, "single column exceeded union budget (impossible)"
        blocks.append((block, sorted(union)))
    return blocks


def _build_plan(diag, off_diag, pp, lp, rp, n):
    """Returns (T, W [128, T*128] f32 in lhsT layout, g_in [T*128] row-gather
    indices into the [2N, B] transposed input, g_fin [2N] row-gather indices
    into the device output)."""
    blocks = _group_columns(pp, n)
    T = len(blocks)
    W = np.zeros((128, T * 128), dtype=np.float32)
    g_in = np.zeros(T * 128, dtype=np.int64)
    pos_of_k = np.zeros(n, dtype=np.int64)
    for tt, (outc, inc) in enumerate(blocks):
        idx = {c: i for i, c in enumerate(inc)}
        for r, col in enumerate(inc):
            g_in[tt * 128 + r] = lp[col]  # component 0 rows
            g_in[tt * 128 + 64 + r] = n + lp[col]  # component 1 rows
        for u, k in enumerate(outc):
            p = int(pp[k])
            ik, ip = idx[k], idx[p]
            po0 = tt * 128 + u
            po1 = tt * 128 + 64 + u
            d0, d1 = float(diag[0, k]), float(diag[1, k])
            f0, f1 = float(off_diag[0, p]), float(off_diag[1, p])
            W[ik, po0] += d0
            W[64 + ik, po0] += -d1
            W[ip, po0] += f0
            W[64 + ip, po0] += -f1
            W[ik, po1] += d1
            W[64 + ik, po1] += d0
            W[ip, po1] += f1
            W[64 + ip, po1] += f0
            pos_of_k[k] = tt * 128 + u
    g_fin = np.empty(2 * n, dtype=np.int64)
    g_fin[:n] = pos_of_k[rp]
    g_fin[n:] = pos_of_k[rp] + 64
    return T, W, g_in, g_fin


def _apply_plan_numpy(W, g_in, g_fin, xt, n):
    """Reference emulation of the device program (for plan validation)."""
    T = W.shape[1] // 128
    dev_in = xt[g_in]
    dev_out = np.empty_like(dev_in)
    for tt in range(T):
        wt = W[:, tt * 128 : (tt + 1) * 128]
        dev_out[tt * 128 : (tt + 1) * 128] = wt.T @ dev_in[tt * 128 : (tt + 1) * 128]
    return dev_out[g_fin]


def _build_program(
    T,
    bc,
    ftile=None,
    bufs=4,
    wsplit=False,
    dma_mode="mixed",
    wengine="scalar",
    in_bufs=None,
    out_bufs=None,
    ramp=False,
    copy_eng="both",
    pe_warm=4,
):
    import concourse.bacc as bacc
    import concourse.bass as bass
    import concourse.mybir as mybir
    import concourse.tile as tile

    ftile = ftile or FTILE
    while bc % ftile:
        ftile //= 2
    assert ftile % MMF == 0 and bc % ftile == 0, (bc, ftile)
    in_bufs = in_bufs or bufs
    out_bufs = out_bufs or bufs

    def widths_for(tt):
        # Uniform ftile-wide positions, except optionally ramped tile widths
        # at the very start (compute/stores begin sooner -> shorter pipeline
        # fill) and the very end (faster drain of the final stores).
        ws = [ftile] * (bc // ftile)
        if ramp and ftile == 2048 and bc >= 2 * ftile:
            if tt == 0:
                ws = [512, 512, 1024] + [ftile] * ((bc - 2048) // ftile)
            elif tt == T - 1 and ramp is True:  # ramp="up" skips the tail ramp
                ws = [ftile] * ((bc - 2048) // ftile) + [1024, 512, 512]
        return ws

    nc = bacc.Bacc("TRN2", target_bir_lowering=False, debug=False)
    R = T * 128
    a = nc.dram_tensor("a", [R, bc], mybir.dt.float32, kind="ExternalInput")
    w = nc.dram_tensor("w", [128, R], mybir.dt.float32, kind="ExternalInput")
    o = nc.dram_tensor("o", [R, bc], mybir.dt.float32, kind="ExternalOutput")

    with tile.TileContext(nc) as tc:
        with (
            tc.tile_pool(name="wpool", bufs=1) as wpool,
            tc.tile_pool(name="inp", bufs=in_bufs) as inp,
            tc.tile_pool(name="outp", bufs=out_bufs) as outp,
            tc.tile_pool(name="ps", bufs=8, space=bass.MemorySpace.PSUM) as ps,
        ):
            w_s = wpool.tile([128, R], mybir.dt.float32)
            # issue the coefficient load on the store ring (idle at startup)
            # so it doesn't delay the first input-tile load on the sync ring
            w_eng = nc.scalar if wengine == "scalar" else nc.sync
            if wsplit:
                # one DMA per W block so the first matmul only waits on
                # its own 64KB block, not the full 1MB coefficient load
                for tt in range(T):
                    w_eng.dma_start(
                        w_s[:, tt * 128 : (tt + 1) * 128],
                        w[:, tt * 128 : (tt + 1) * 128],
                    )
            else:
                w_eng.dma_start(w_s[:], w[:])
            if pe_warm:
                # Dummy matmuls on a zeroed SBUF tile during the DMA fill
                # window: releases the PE HAM clock-gate (~4us of sustained
                # activity -> full 2.4 GHz) before the first real matmul,
                # at zero HBM cost.
                zt = wpool.tile([128, 512], mybir.dt.float32, tag="pewarm")
                nc.gpsimd.memset(zt[:], 0.0)
                pwt = ps.tile([128, MMF], mybir.dt.float32, tag="pt")
                for _ in range(pe_warm):
                    nc.tensor.matmul(
                        pwt[:], zt[:, :128], zt[:, :MMF], start=True, stop=True
                    )
            pos = 0
            for tt in range(T):
                wt = w_s[:, tt * 128 : (tt + 1) * 128]
                c0 = 0
                for width in widths_for(tt):
                    # which HWDGE ring (sync vs scalar engine) issues each DMA
                    if dma_mode == "spread":
                        ld_eng = nc.sync if pos % 2 == 0 else nc.scalar
                        st_eng = nc.scalar if pos % 2 == 0 else nc.sync
                    elif dma_mode == "sync":
                        ld_eng = st_eng = nc.sync
                    else:  # "mixed": loads on sync, stores on scalar
                        ld_eng, st_eng = nc.sync, nc.scalar
                    pos += 1
                    tin = inp.tile([128, width], mybir.dt.float32)
                    ld_eng.dma_start(
                        tin[:],
                        a[tt * 128 : (tt + 1) * 128, c0 : c0 + width],
                    )
                    tout = outp.tile([128, width], mybir.dt.float32)
                    for q in range(width // MMF):
                        pt = ps.tile([128, MMF], mybir.dt.float32)
                        nc.tensor.matmul(
                            pt[:],
                            wt,
                            tin[:, q * MMF : (q + 1) * MMF],
                            start=True,
                            stop=True,
                        )
                        if copy_eng == "dve" or (copy_eng == "both" and q % 2 == 0):
                            nc.vector.tensor_copy(tout[:, q * MMF : (q + 1) * MMF], pt[:])
                        else:
                            nc.scalar.copy(tout[:, q * MMF : (q + 1) * MMF], pt[:])
                    st_eng.dma_start(
                        o[tt * 128 : (tt + 1) * 128, c0 : c0 + width],
                        tout[:],
                    )
                    c0 += width
    nc.compile()
    return nc


# Overridable program-variant knobs (used by experiment sweeps).
PROG_KWARGS: dict = {}


def _get_program(T, bc):
    key = (T, bc, tuple(sorted(PROG_KWARGS.items())))
    if key not in _prog_cache:
        _prog_cache[key] = _build_program(T, bc, **PROG_KWARGS)
    return _prog_cache[key]


def kernel(x, diag, off_diag, pairwise_perm_idx, left_perm_idx, right_perm_idx):
    global LAST_RESULTS
    from concourse.bass_utils import run_bass_kernel_spmd

    x = np.asarray(x)
    in_dtype = x.dtype
    diag = np.asarray(diag, dtype=np.float32)
    off_diag = np.asarray(off_diag, dtype=np.float32)
    pp = np.asarray(pairwise_perm_idx, dtype=np.int64)
    lp = np.asarray(left_perm_idx, dtype=np.int64)
    rp = np.asarray(right_perm_idx, dtype=np.int64)
    _, B, n = x.shape
    bc = B // NCORES

    T, W, g_in, g_fin = _build_plan(diag, off_diag, pp, lp, rp, n)

    # Host-side: transpose to [2N, B] and pre-gather rows into tile order.
    xt = np.ascontiguousarray(x.astype(np.float32, copy=False).transpose(0, 2, 1)).reshape(
        2 * n, B
    )
    dev_in = xt[g_in]  # [T*128, B]

    nc = _get_program(T, bc)
    in_maps = [
        {"a": np.ascontiguousarray(dev_in[:, c * bc : (c + 1) * bc]), "w": W}
        for c in range(NCORES)
    ]
    LAST_RESULTS = run_bass_kernel_spmd(nc, in_maps, list(range(NCORES)))
    dev_out = np.concatenate([LAST_RESULTS.results[c]["o"] for c in range(NCORES)], axis=1)

    out = dev_out[g_fin].reshape(2, n, B).transpose(0, 2, 1)
    return np.ascontiguousarray(out).astype(in_dtype, copy=False)
